# revision 17
# baseline (speedup 1.0000x reference)
"""Trainium2 Bass kernel for a differentiable GRU decoder — fp8 DoubleRow version.

Per step t (max_len=32 steps), batch N=4096, E=512, V=1024:
    emb    = probs_{t-1} @ W_d2e.T            # [N, E]
    h      = GRUCell(emb, h)                  # [N, E]
    logits = h @ W_e2d.T + b_e2d              # [N, V]
    probs  = softmax(logits)                  # [N, V]  -> output[t]

Sharding: data-parallel over N across 8 cores (512 rows each), weights
replicated, the 32-step scan stays local per core — no collectives.

v2 design (vs the bf16 baseline):
- All matmuls except logits run as fp8(e4m3) DoubleRow (256-contraction
  per instruction, ~2x bf16 MAC throughput on HW).  Numerics validated
  in sim: logits path must stay bf16 (h2/W_e2d); everything else e4m3.
- Power-of-2 scales keep every fp8 tensor in e4m3's normal range AND
  make the gh and gx halves share one PSUM accumulation group:
      h_q = h*32, W_hh*128  -> gh psum scale 2^12
      emb_q = emb*256, W_ih*16 -> gx psum scale 2^12
  The sigmoid/tanh drains undo the scale via the ACT `scale` operand.
- The device streams RAW fp16 logits (no bias); the host adds b_e2d and
  does the softmax.  This removes the bf16-exp output floor (~4e-3) and
  the on-device out-normalization entirely.
- exp(logits) is written once, as the fp8 eT operand of the next emb
  matmul; the row-sum (partition reduction) is a fp8 DoubleRow matmul
  against a 2^-4 ones matrix, giving rbc = 16/S which folds the 2^8
  emb scale into the drain: emb_q = psum(=16*S*emb) * rbc = 256*emb.
- GRU tail keeps the baseline tricks: gh emitted before gx in each
  PSUM group, (1-z) via a negated sigmoid drain, z*h on the Pool
  engine off the critical path, fp32 h master deferred one step.
"""

import os
import sys
import types

import numpy as np

import concourse.bacc as bacc
import concourse.mybir as mybir
import concourse.tile as tile

F32 = mybir.dt.float32
BF16 = mybir.dt.bfloat16
FP16 = mybir.dt.float16
FP8 = mybir.dt.float8e4
AF = mybir.ActivationFunctionType
DR = mybir.MatmulPerfMode.DoubleRow
ALU = mybir.AluOpType

N_CORES = 8

SH = 32.0        # h_q scale (|h| <= ~5.1 early; 5.1*32 < 240)
SWHH = 128.0     # W_hh scale -> gh psum 2^12
SEMB = 256.0     # emb_q scale (|emb| <= 1/32; *256 <= 8)
SWIH = 16.0      # W_ih scale -> gx psum 2^12
SWD2E = 16.0     # W_d2e scale
SGI = 1.0 / 4096.0   # gate drain scale 2^-12
SONES = 1.0 / 16.0   # ones value: rowsum psum = S/16, rbc = 16/S


def _install_ntff_hook():
    """Register the axon NTFF profiling hook if the image's antenv lacks it."""
    try:
        import antenv.axon_hooks  # noqa: F401
        return
    except ImportError:
        pass
    try:
        from trn_agent_boot.trn_boot import _ntff_profile_via_ctypes

        hook = _ntff_profile_via_ctypes("/opt/axon/libaxon_pjrt.so")
    except Exception:
        hook = None
    mod = types.ModuleType("antenv.axon_hooks")
    mod.get_axon_ntff_profile_hook = lambda: hook
    mod.set_axon_ntff_profile_hook = lambda h: None
    sys.modules["antenv.axon_hooks"] = mod


_install_ntff_hook()


def _build(T, B, E, V):
    """Build the per-core Bacc module. B = per-core batch (free dim)."""
    KE = E // 128   # 4 E-blocks
    KV = V // 128   # 8 V-blocks
    PH = KE // 2    # 2 contraction pairs over E
    PV = KV // 2    # 4 contraction pairs over V

    nc = bacc.Bacc(None, target_bir_lowering=False)

    xT = nc.dram_tensor("xT", [E, B], F32, kind="ExternalInput")
    whh8 = [
        nc.dram_tensor(f"whh8_{p}", [128, 2, 3 * E], FP8, kind="ExternalInput")
        for p in range(PH)
    ]
    wih8 = [
        nc.dram_tensor(f"wih8_{p}", [128, 2, 3 * E], FP8, kind="ExternalInput")
        for p in range(PH)
    ]
    wd2e8 = [
        nc.dram_tensor(f"wd2e8_{p}", [128, 2, E], FP8, kind="ExternalInput")
        for p in range(PV)
    ]
    we2dT = nc.dram_tensor("we2dT", [E, V], BF16, kind="ExternalInput")
    # b_rz: cols [0:2*KE) = (b_ih+b_hh) for r,z; cols [2*KE:3*KE) = negated z part
    brz = nc.dram_tensor("brz", [128, 3 * KE], F32, kind="ExternalInput")
    bihn = nc.dram_tensor("bihn", [128, KE], F32, kind="ExternalInput")
    bhhn12 = nc.dram_tensor("bhhn12", [128, KE], F32, kind="ExternalInput")
    be2d = nc.dram_tensor("be2d", [128, KV], F32, kind="ExternalInput")
    # raw fp16 logits, NO bias; host adds b_e2d and softmaxes
    out_l = nc.dram_tensor("out_l", [T, V, B], FP16, kind="ExternalOutput")

    with tile.TileContext(nc) as tc:
        with (
            tc.tile_pool(name="w", bufs=1) as wp,
            tc.tile_pool(name="sb", bufs=1) as sb,
            tc.tile_pool(name="ps", bufs=1, space="PSUM") as pp,
        ):
            # ---- initial state h = x, loaded first (t=0 gh needs h_q);
            # x rides the SWDGE queues to avoid serializing on weight DMAs
            hT = []
            for m in range(KE):
                hf = sb.tile([128, B], F32, name="h", tag="h", bufs=8)
                nc.gpsimd.dma_start(hf[:], xT[m * 128 : (m + 1) * 128, :])
                hT.append(hf)
            hq = [
                sb.tile([128, 2, B], FP8, name="hq", tag="hq", bufs=4)
                for _ in range(PH)
            ]
            for m in range(KE):
                nc.scalar.activation(
                    hq[m // 2][:, m % 2, :], hT[m][:], AF.Copy, scale=SH
                )

            # ---- persistent weights, in first-use order ----
            w_hh = []
            for p in range(PH):
                wt = wp.tile([128, 2, 3 * E], FP8, name=f"w_hh{p}", tag=f"w_hh{p}")
                nc.sync.dma_start(wt[:], whh8[p][:])
                w_hh.append(wt)
            b_rz = wp.tile([128, 3 * KE], F32, name="b_rz", tag="b_rz")
            nc.sync.dma_start(b_rz[:], brz[:])
            b_ihn = wp.tile([128, KE], F32, name="b_ihn", tag="b_ihn")
            nc.sync.dma_start(b_ihn[:], bihn[:])
            b_hhn12 = wp.tile([128, KE], F32, name="b_hhn12", tag="b_hhn12")
            nc.sync.dma_start(b_hhn12[:], bhhn12[:])
            b_e2d = wp.tile([128, KV], F32, name="b_e2d", tag="b_e2d")
            nc.sync.dma_start(b_e2d[:], be2d[:])
            w_e2d = []
            for k in range(KE):
                wt = wp.tile([128, V], BF16, name=f"w_e2d{k}", tag=f"w_e2d{k}")
                nc.sync.dma_start(wt[:], we2dT[k * 128 : (k + 1) * 128, :])
                w_e2d.append(wt)
            w_d2e = []
            for p in range(PV):
                wt = wp.tile([128, 2, E], FP8, name=f"w_d2e{p}", tag=f"w_d2e{p}")
                nc.sync.dma_start(wt[:], wd2e8[p][:])
                w_d2e.append(wt)
            w_ih = []
            for p in range(PH):
                wt = wp.tile([128, 2, 3 * E], FP8, name=f"w_ih{p}", tag=f"w_ih{p}")
                nc.sync.dma_start(wt[:], wih8[p][:])
                w_ih.append(wt)

            tbl_junk = wp.tile([128, 1], F32, name="tbl_junk", tag="tbl_junk")
            ones_f32 = wp.tile([128, 2, 128], F32, name="ones_f32", tag="ones_f32")
            nc.gpsimd.memset(ones_f32[:], SONES)
            ones8 = wp.tile([128, 2, 128], FP8, name="ones8", tag="ones8")
            nc.scalar.activation(ones8[:], ones_f32[:], AF.Copy)

            eTq = None       # fp8 exp pairs of previous step
            rbc = None       # 16/rowsum broadcast [128, B] fp32
            hf_pending = None  # (t2, zh) pairs for the deferred fp32 h master

            for t in range(T):
                # ---- emb_q = (softmax_{t-1} @ W_d2e.T) * 256, via fp8 DR;
                # normalization + scales folded into the rbc drain ----
                embq = None
                if t > 0:
                    embq = []
                    for half in range(PH):
                        e16 = sb.tile(
                            [128, 2 * B], BF16, name="emb16", tag="emb16", bufs=4
                        )
                        for ml in range(2):
                            m = half * 2 + ml
                            ps = pp.tile(
                                [128, B], F32, name="ps_mm", tag="mm", bufs=8
                            )
                            for jp in range(PV):
                                nc.tensor.matmul(
                                    ps[:],
                                    w_d2e[jp][:, :, m * 128 : (m + 1) * 128],
                                    eTq[jp][:],
                                    start=(jp == 0),
                                    stop=(jp == PV - 1),
                                    perf_mode=DR,
                                )
                            nc.vector.tensor_mul(
                                e16[:, ml * B : (ml + 1) * B], ps[:], rbc[:]
                            )
                        eq = sb.tile(
                            [128, 2, B], FP8, name="embq", tag="embq", bufs=4
                        )
                        nc.scalar.activation(
                            eq[:], e16[:].rearrange("p (a b) -> p a b", a=2),
                            AF.Copy,
                        )
                        embq.append(eq)

                # ---- gates r, z: sigmoid(2^-12 psum + b); gh pairs emitted
                # first so the PE has emb-independent work during the tail.
                # z additionally drains (1-z) via sigmoid(-x, -b) ----
                r_g = []
                z_g = []
                omz_g = []
                for g in range(2):
                    for m in range(KE):
                        col = g * E + m * 128
                        ps = pp.tile([128, B], F32, name="ps_mm", tag="mm", bufs=8)
                        for p in range(PH):
                            nc.tensor.matmul(
                                ps[:],
                                w_hh[p][:, :, col : col + 128],
                                hq[p][:],
                                start=(p == 0),
                                stop=(t == 0 and p == PH - 1),
                                perf_mode=DR,
                            )
                        if t > 0:
                            for p in range(PH):
                                nc.tensor.matmul(
                                    ps[:],
                                    w_ih[p][:, :, col : col + 128],
                                    embq[p][:],
                                    start=False,
                                    stop=(p == PH - 1),
                                    perf_mode=DR,
                                )
                        j = g * KE + m
                        if g == 0:
                            gt = sb.tile(
                                [128, B], F32, name="gate_r", tag="gate_r", bufs=4
                            )
                            nc.scalar.activation(
                                gt[:], ps[:], AF.Sigmoid,
                                bias=b_rz[:, j : j + 1], scale=SGI,
                            )
                            r_g.append(gt)
                        else:
                            zt = sb.tile(
                                [128, B], F32, name="gate_z", tag="gate_z", bufs=4
                            )
                            nc.scalar.activation(
                                zt[:], ps[:], AF.Sigmoid,
                                bias=b_rz[:, j : j + 1], scale=SGI,
                            )
                            oz = sb.tile(
                                [128, B], F32, name="gate_omz", tag="gate_omz",
                                bufs=4,
                            )
                            nj = 2 * KE + m
                            nc.scalar.activation(
                                oz[:], ps[:], AF.Sigmoid,
                                bias=b_rz[:, nj : nj + 1], scale=-SGI,
                            )
                            omz_g.append(oz)
                            z_g.append(zt)

                # fp32 h master of the PREVIOUS step, deferred past the gate
                # matmuls so their hoisted waits never include these DVE ops
                if hf_pending is not None:
                    hN = []
                    for m in range(KE):
                        t2p, zhp = hf_pending[m]
                        hf = sb.tile([128, B], F32, name="h", tag="h", bufs=8)
                        nc.vector.tensor_add(hf[:], t2p[:], zhp[:])
                        hN.append(hf)
                    hT = hN
                    hf_pending = None

                # ---- n gate: tanh(2^-12*(ghn + 2^12*b_hhn)*r-ish + b_ihn);
                # the (hn + b_hhn)*r combine is one DVE scalar_tensor_tensor ----
                t2_g = []
                for m in range(KE):
                    col = 2 * E + m * 128
                    ps = pp.tile([128, B], F32, name="ps_mm", tag="mm", bufs=8)
                    for p in range(PH):
                        nc.tensor.matmul(
                            ps[:],
                            w_hh[p][:, :, col : col + 128],
                            hq[p][:],
                            start=(p == 0),
                            stop=(p == PH - 1),
                            perf_mode=DR,
                        )
                    t2 = sb.tile([128, B], F32, name="t2", tag="t2", bufs=8)
                    nc.vector.scalar_tensor_tensor(
                        t2[:], ps[:], b_hhn12[:, m : m + 1], r_g[m][:],
                        op0=ALU.add, op1=ALU.mult,
                    )
                    t2_g.append(t2)

                if t > 0:
                    for m in range(KE):
                        col = 2 * E + m * 128
                        ps = pp.tile([128, B], F32, name="ps_mm", tag="mm", bufs=8)
                        for p in range(PH):
                            nc.tensor.matmul(
                                ps[:],
                                w_ih[p][:, :, col : col + 128],
                                embq[p][:],
                                start=(p == 0),
                                stop=(p == PH - 1),
                                perf_mode=DR,
                            )
                        nc.vector.tensor_add(t2_g[m][:], t2_g[m][:], ps[:])

                # z*h on the Pool engine, off the critical path
                zh_g = []
                for m in range(KE):
                    zh = sb.tile([128, B], F32, name="zh", tag="zh", bufs=8)
                    nc.gpsimd.tensor_mul(zh[:], z_g[m][:], hT[m][:])
                    zh_g.append(zh)

                # ---- h' = (1-z)*n + z*h; bf16 h2 (logits operand) written
                # first, fp8 h_q cast from h2, fp32 master deferred ----
                last = t == T - 1
                if not last:
                    hq = [
                        sb.tile([128, 2, B], FP8, name="hq", tag="hq", bufs=4)
                        for _ in range(PH)
                    ]
                h2 = []
                hf_pending = []
                for m in range(KE):
                    t2 = t2_g[m]
                    nc.scalar.activation(
                        t2[:], t2[:], AF.Tanh,
                        bias=b_ihn[:, m : m + 1], scale=SGI,
                    )  # n, in place
                    nc.vector.tensor_mul(t2[:], t2[:], omz_g[m][:])  # (1-z)*n
                    hm = sb.tile([128, B], BF16, name="h2", tag="h2", bufs=8)
                    nc.vector.tensor_add(hm[:], t2[:], zh_g[m][:])
                    h2.append(hm)
                    if not last:
                        nc.scalar.activation(
                            hq[m // 2][:, m % 2, :], hm[:], AF.Copy, scale=SH
                        )
                    hf_pending.append((t2, zh_g[m]))

                # [128,1] dummy exp: hoists the sigmoid->exp ACT table
                # load off the tail (it runs while the logits matmuls fill)
                nc.scalar.activation(tbl_junk[:], b_e2d[:, 0:1], AF.Exp)

                # dependency-free warm-up matmuls into the j=0 logits bank:
                # they run the moment the PE drains its real queue, keeping
                # the p-state up through the GRU-tail idle window.  The real
                # k=0 matmul below has start=True, which wipes the junk.
                ps0 = pp.tile([128, B], F32, name="ps_mm", tag="mm", bufs=8)
                for _ in range(12):
                    nc.tensor.matmul(
                        ps0[:],
                        w_e2d[1][:, 0:128],
                        w_e2d[0][:, 0:B],
                        start=True,
                        stop=True,
                    )

                # ---- logits psum (bf16 matmuls); exp -> fp8 eT pairs for the
                # next emb matmul; raw fp16 logits stream out (host adds bias
                # + softmax); rowsum via fp8 DR ones matmul -> rbc = 16/S ----
                if not last:
                    eTq = [
                        sb.tile([128, 2, B], FP8, name="eT", tag="eT", bufs=8)
                        for _ in range(PV)
                    ]
                    ps_s = pp.tile([128, B], F32, name="ps_s", tag="mm", bufs=8)
                for j in range(KV):
                    ps = ps0 if j == 0 else pp.tile(
                        [128, B], F32, name="ps_mm", tag="mm", bufs=8
                    )
                    for k in range(KE):
                        nc.tensor.matmul(
                            ps[:],
                            w_e2d[k][:, j * 128 : (j + 1) * 128],
                            h2[k][:],
                            start=(k == 0),
                            stop=(k == KE - 1),
                        )
                    if not last:
                        nc.scalar.activation(
                            eTq[j // 2][:, j % 2, :], ps[:], AF.Exp,
                            bias=b_e2d[:, j : j + 1],
                        )
                    lo = sb.tile([128, B], FP16, name="lo", tag="lo", bufs=16)
                    nc.vector.tensor_copy(lo[:], ps[:])
                    nc.sync.dma_start(out_l[t, j * 128 : (j + 1) * 128, :], lo[:])
                    if not last and j % 2 == 1:
                        jp = j // 2
                        nc.tensor.matmul(
                            ps_s[:],
                            ones8[:],
                            eTq[jp][:],
                            start=(jp == 0),
                            stop=(jp == PV - 1),
                            perf_mode=DR,
                        )
                # [128,1] dummy sigmoid: hoists the exp->sigmoid table
                # load into the rowsum/recip window of this step's tail
                nc.scalar.activation(tbl_junk[:], b_e2d[:, 0:1], AF.Sigmoid)
                if not last:
                    rbc = sb.tile([128, B], F32, name="rbc", tag="rbc", bufs=2)
                    nc.vector.reciprocal_approx_fast(rbc[:], ps_s[:])

    nc.compile()
    return nc


def _prep_inputs(x, W_d2e, W_ih, W_hh, b_ih, b_hh, W_e2d, b_e2d):
    import ml_dtypes

    E = x.shape[1]
    V = np.asarray(W_e2d).shape[0]
    KE = E // 128
    KV = V // 128
    PH = KE // 2
    PV = KV // 2
    E4 = ml_dtypes.float8_e4m3
    BFN = ml_dtypes.bfloat16

    def c(a, dt=np.float32):
        return np.ascontiguousarray(np.asarray(a, dtype=np.float32).astype(dt))

    def pairs(WT, scale, npairs):
        # WT: [contraction, cols] -> list of [128, 2, cols] fp8, pair p holds
        # contraction rows [p*256+i*128+k]
        cols = WT.shape[1]
        a = (np.asarray(WT, dtype=np.float32) * scale).reshape(
            npairs, 2, 128, cols
        )
        return [c(a[p].transpose(1, 0, 2), E4) for p in range(npairs)]

    b_ih = np.asarray(b_ih, dtype=np.float32)
    b_hh = np.asarray(b_hh, dtype=np.float32)
    brz_sum = (b_ih + b_hh)[: 2 * E].reshape(2 * KE, 128).T   # [128, 8]
    brz_negz = -(b_ih + b_hh)[E : 2 * E].reshape(KE, 128).T   # [128, 4]

    whh_p = pairs(np.asarray(W_hh).T, SWHH, PH)
    wih_p = pairs(np.asarray(W_ih).T, SWIH, PH)
    wd2e_p = pairs(np.asarray(W_d2e).T, SWD2E, PV)

    shared = {
        "we2dT": c(np.asarray(W_e2d).T, BFN),                  # [E, V]
        "brz": c(np.concatenate([brz_sum, brz_negz], axis=1)),  # [128, 12]
        "bihn": c(b_ih[2 * E :].reshape(KE, 128).T),
        "bhhn12": c(b_hh[2 * E :].reshape(KE, 128).T * 4096.0),
        "be2d": c(np.asarray(b_e2d).reshape(KV, 128).T),
    }
    for p in range(PH):
        shared[f"whh8_{p}"] = whh_p[p]
        shared[f"wih8_{p}"] = wih_p[p]
    for p in range(PV):
        shared[f"wd2e8_{p}"] = wd2e_p[p]

    N = x.shape[0]
    B = N // N_CORES
    in_maps = []
    for core in range(N_CORES):
        m = dict(shared)
        m["xT"] = c(np.asarray(x)[core * B : (core + 1) * B, :].T)  # [E, B]
        in_maps.append(m)
    return in_maps, B


def _run(inputs, trace=False):
    from concourse.bass_utils import run_bass_kernel_spmd

    x = np.asarray(inputs["x"], dtype=np.float32)
    T = int(inputs["max_len"])
    N, E = x.shape
    V = np.asarray(inputs["W_e2d"]).shape[0]
    assert N % N_CORES == 0 and E % 256 == 0 and V % 256 == 0

    in_maps, B = _prep_inputs(
        x,
        inputs["W_d2e"],
        inputs["W_ih"],
        inputs["W_hh"],
        inputs["b_ih"],
        inputs["b_hh"],
        inputs["W_e2d"],
        inputs["b_e2d"],
    )
    nc = _build(T, B, E, V)
    res = run_bass_kernel_spmd(
        nc, in_maps, core_ids=list(range(N_CORES)), trace=trace
    )

    b_e2d = np.asarray(inputs["b_e2d"], dtype=np.float32)
    full = np.empty((T, N, V), dtype=np.float32)
    for core in range(N_CORES):
        l = np.asarray(res.results[core]["out_l"], dtype=np.float32)  # [T, V, B]
        l += b_e2d[None, :, None]
        l -= l.max(axis=1, keepdims=True)
        e = np.exp(l)
        e /= e.sum(axis=1, keepdims=True)
        full[:, core * B : (core + 1) * B, :] = np.transpose(e, (0, 2, 1))
    return full, res


def kernel(**inputs):
    full, _ = _run(inputs, trace=False)
    return full


def run_traced(**inputs):
    return _run(inputs, trace=True)


# revision 18
# speedup vs baseline: 1.1885x; 1.1885x over previous
"""Trainium2 Bass kernel for a differentiable GRU decoder — fp8 DoubleRow version.

Per step t (max_len=32 steps), batch N=4096, E=512, V=1024:
    emb    = probs_{t-1} @ W_d2e.T            # [N, E]
    h      = GRUCell(emb, h)                  # [N, E]
    logits = h @ W_e2d.T + b_e2d              # [N, V]
    probs  = softmax(logits)                  # [N, V]  -> output[t]

Sharding: data-parallel over N across 8 cores (512 rows each), weights
replicated, the 32-step scan stays local per core — no collectives.

v2 design (vs the bf16 baseline):
- All matmuls except logits run as fp8(e4m3) DoubleRow (256-contraction
  per instruction, ~2x bf16 MAC throughput on HW).  Numerics validated
  in sim: logits path must stay bf16 (h2/W_e2d); everything else e4m3.
- Power-of-2 scales keep every fp8 tensor in e4m3's normal range AND
  make the gh and gx halves share one PSUM accumulation group:
      h_q = h*32, W_hh*128  -> gh psum scale 2^12
      emb_q = emb*256, W_ih*16 -> gx psum scale 2^12
  The sigmoid/tanh drains undo the scale via the ACT `scale` operand.
- The device streams RAW fp16 logits (no bias); the host adds b_e2d and
  does the softmax.  This removes the bf16-exp output floor (~4e-3) and
  the on-device out-normalization entirely.
- exp(logits) is written once, as the fp8 eT operand of the next emb
  matmul; the row-sum (partition reduction) is a fp8 DoubleRow matmul
  against a 2^-4 ones matrix, giving rbc = 16/S which folds the 2^8
  emb scale into the drain: emb_q = psum(=16*S*emb) * rbc = 256*emb.
- GRU tail keeps the baseline tricks: gh emitted before gx in each
  PSUM group, (1-z) via a negated sigmoid drain, z*h on the Pool
  engine off the critical path, fp32 h master deferred one step.
"""

import os
import sys
import types

import numpy as np

import concourse.bacc as bacc
import concourse.mybir as mybir
import concourse.tile as tile

F32 = mybir.dt.float32
BF16 = mybir.dt.bfloat16
FP16 = mybir.dt.float16
FP8 = mybir.dt.float8e4
AF = mybir.ActivationFunctionType
DR = mybir.MatmulPerfMode.DoubleRow
ALU = mybir.AluOpType

N_CORES = 8

SH = 32.0        # h_q scale (|h| <= ~5.1 early; 5.1*32 < 240)
SWHH = 128.0     # W_hh scale -> gh psum 2^12
SEMB = 256.0     # emb_q scale (|emb| <= 1/32; *256 <= 8)
SWIH = 16.0      # W_ih scale -> gx psum 2^12
SWD2E = 16.0     # W_d2e scale
SGI = 1.0 / 4096.0   # gate drain scale 2^-12
SONES = 1.0 / 16.0   # ones value: rowsum psum = S/16, rbc = 16/S


def _install_ntff_hook():
    """Register the axon NTFF profiling hook if the image's antenv lacks it."""
    try:
        import antenv.axon_hooks  # noqa: F401
        return
    except ImportError:
        pass
    try:
        from trn_agent_boot.trn_boot import _ntff_profile_via_ctypes

        hook = _ntff_profile_via_ctypes("/opt/axon/libaxon_pjrt.so")
    except Exception:
        hook = None
    mod = types.ModuleType("antenv.axon_hooks")
    mod.get_axon_ntff_profile_hook = lambda: hook
    mod.set_axon_ntff_profile_hook = lambda h: None
    sys.modules["antenv.axon_hooks"] = mod


_install_ntff_hook()


def _build(T, B, E, V):
    """Build the per-core Bacc module. B = per-core batch (free dim)."""
    KE = E // 128   # 4 E-blocks
    KV = V // 128   # 8 V-blocks
    PH = KE // 2    # 2 contraction pairs over E
    PV = KV // 2    # 4 contraction pairs over V

    nc = bacc.Bacc(None, target_bir_lowering=False)

    xT = nc.dram_tensor("xT", [E, B], F32, kind="ExternalInput")
    whh8 = [
        nc.dram_tensor(f"whh8_{p}", [128, 2, 3 * E], FP8, kind="ExternalInput")
        for p in range(PH)
    ]
    wih8 = [
        nc.dram_tensor(f"wih8_{p}", [128, 2, 3 * E], FP8, kind="ExternalInput")
        for p in range(PH)
    ]
    wd2e8 = [
        nc.dram_tensor(f"wd2e8_{p}", [128, 2, E], FP8, kind="ExternalInput")
        for p in range(PV)
    ]
    we2dT = nc.dram_tensor("we2dT", [E, V], BF16, kind="ExternalInput")
    # b_rz: cols [0:2*KE) = (b_ih+b_hh) for r,z; cols [2*KE:3*KE) = negated z part
    brz = nc.dram_tensor("brz", [128, 3 * KE], F32, kind="ExternalInput")
    bihn = nc.dram_tensor("bihn", [128, KE], F32, kind="ExternalInput")
    bhhn12 = nc.dram_tensor("bhhn12", [128, KE], F32, kind="ExternalInput")
    be2d = nc.dram_tensor("be2d", [128, KV], F32, kind="ExternalInput")
    # raw fp16 logits, NO bias; host adds b_e2d and softmaxes
    out_l = nc.dram_tensor("out_l", [T, V, B], FP16, kind="ExternalOutput")

    with tile.TileContext(nc) as tc:
        with (
            tc.tile_pool(name="w", bufs=1) as wp,
            tc.tile_pool(name="sb", bufs=1) as sb,
            tc.tile_pool(name="ps", bufs=1, space="PSUM") as pp,
        ):
            # ---- initial state h = x, loaded first (t=0 gh needs h_q);
            # x rides the SWDGE queues to avoid serializing on weight DMAs
            hT = []
            for m in range(KE):
                hf = sb.tile([128, B], F32, name="h", tag="h", bufs=8)
                nc.gpsimd.dma_start(hf[:], xT[m * 128 : (m + 1) * 128, :])
                hT.append(hf)
            hq = [
                sb.tile([128, 2, B], FP8, name="hq", tag="hq", bufs=4)
                for _ in range(PH)
            ]
            for m in range(KE):
                nc.scalar.activation(
                    hq[m // 2][:, m % 2, :], hT[m][:], AF.Copy, scale=SH
                )

            # ---- persistent weights, in first-use order ----
            w_hh = []
            for p in range(PH):
                wt = wp.tile([128, 2, 3 * E], FP8, name=f"w_hh{p}", tag=f"w_hh{p}")
                nc.sync.dma_start(wt[:], whh8[p][:])
                w_hh.append(wt)
            b_rz = wp.tile([128, 3 * KE], F32, name="b_rz", tag="b_rz")
            nc.sync.dma_start(b_rz[:], brz[:])
            b_ihn = wp.tile([128, KE], F32, name="b_ihn", tag="b_ihn")
            nc.sync.dma_start(b_ihn[:], bihn[:])
            b_hhn12 = wp.tile([128, KE], F32, name="b_hhn12", tag="b_hhn12")
            nc.sync.dma_start(b_hhn12[:], bhhn12[:])
            b_e2d = wp.tile([128, KV], F32, name="b_e2d", tag="b_e2d")
            nc.sync.dma_start(b_e2d[:], be2d[:])
            w_e2d = []
            for k in range(KE):
                wt = wp.tile([128, V], BF16, name=f"w_e2d{k}", tag=f"w_e2d{k}")
                nc.sync.dma_start(wt[:], we2dT[k * 128 : (k + 1) * 128, :])
                w_e2d.append(wt)
            w_d2e = []
            for p in range(PV):
                wt = wp.tile([128, 2, E], FP8, name=f"w_d2e{p}", tag=f"w_d2e{p}")
                nc.sync.dma_start(wt[:], wd2e8[p][:])
                w_d2e.append(wt)
            w_ih = []
            for p in range(PH):
                wt = wp.tile([128, 2, 3 * E], FP8, name=f"w_ih{p}", tag=f"w_ih{p}")
                nc.sync.dma_start(wt[:], wih8[p][:])
                w_ih.append(wt)

            ones_f32 = wp.tile([128, 2, 128], F32, name="ones_f32", tag="ones_f32")
            nc.gpsimd.memset(ones_f32[:], SONES)
            ones8 = wp.tile([128, 2, 128], FP8, name="ones8", tag="ones8")
            nc.scalar.activation(ones8[:], ones_f32[:], AF.Copy)

            eTq = None       # fp8 exp pairs of previous step
            rbc = None       # 16/rowsum broadcast [128, B] fp32
            hf_pending = None  # (t2, zh) pairs for the deferred fp32 h master

            for t in range(T):
                # ---- emb_q = (softmax_{t-1} @ W_d2e.T) * 256, via fp8 DR;
                # normalization + scales folded into the rbc drain ----
                embq = None
                if t > 0:
                    embq = []
                    for half in range(PH):
                        e16 = sb.tile(
                            [128, 2 * B], BF16, name="emb16", tag="emb16", bufs=4
                        )
                        for ml in range(2):
                            m = half * 2 + ml
                            ps = pp.tile(
                                [128, B], F32, name="ps_mm", tag="mm", bufs=8
                            )
                            for jp in range(PV):
                                nc.tensor.matmul(
                                    ps[:],
                                    w_d2e[jp][:, :, m * 128 : (m + 1) * 128],
                                    eTq[jp][:],
                                    start=(jp == 0),
                                    stop=(jp == PV - 1),
                                    perf_mode=DR,
                                )
                            nc.vector.tensor_mul(
                                e16[:, ml * B : (ml + 1) * B], ps[:], rbc[:]
                            )
                        eq = sb.tile(
                            [128, 2, B], FP8, name="embq", tag="embq", bufs=4
                        )
                        nc.scalar.activation(
                            eq[:], e16[:].rearrange("p (a b) -> p a b", a=2),
                            AF.Copy,
                        )
                        embq.append(eq)

                # ---- gates r, z: sigmoid(2^-12 psum + b); gh pairs emitted
                # first so the PE has emb-independent work during the tail.
                # z additionally drains (1-z) via sigmoid(-x, -b) ----
                r_g = []
                z_g = []
                omz_g = []
                for g in range(2):
                    for m in range(KE):
                        col = g * E + m * 128
                        ps = pp.tile([128, B], F32, name="ps_mm", tag="mm", bufs=8)
                        for p in range(PH):
                            nc.tensor.matmul(
                                ps[:],
                                w_hh[p][:, :, col : col + 128],
                                hq[p][:],
                                start=(p == 0),
                                stop=(t == 0 and p == PH - 1),
                                perf_mode=DR,
                            )
                        if t > 0:
                            for p in range(PH):
                                nc.tensor.matmul(
                                    ps[:],
                                    w_ih[p][:, :, col : col + 128],
                                    embq[p][:],
                                    start=False,
                                    stop=(p == PH - 1),
                                    perf_mode=DR,
                                )
                        j = g * KE + m
                        if g == 0:
                            gt = sb.tile(
                                [128, B], F32, name="gate_r", tag="gate_r", bufs=4
                            )
                            nc.scalar.activation(
                                gt[:], ps[:], AF.Sigmoid,
                                bias=b_rz[:, j : j + 1], scale=SGI,
                            )
                            r_g.append(gt)
                        else:
                            zt = sb.tile(
                                [128, B], F32, name="gate_z", tag="gate_z", bufs=4
                            )
                            nc.scalar.activation(
                                zt[:], ps[:], AF.Sigmoid,
                                bias=b_rz[:, j : j + 1], scale=SGI,
                            )
                            oz = sb.tile(
                                [128, B], F32, name="gate_omz", tag="gate_omz",
                                bufs=4,
                            )
                            nj = 2 * KE + m
                            nc.scalar.activation(
                                oz[:], ps[:], AF.Sigmoid,
                                bias=b_rz[:, nj : nj + 1], scale=-SGI,
                            )
                            omz_g.append(oz)
                            z_g.append(zt)

                # fp32 h master of the PREVIOUS step, deferred past the gate
                # matmuls so their hoisted waits never include these DVE ops
                if hf_pending is not None:
                    hN = []
                    for m in range(KE):
                        t2p, zhp = hf_pending[m]
                        hf = sb.tile([128, B], F32, name="h", tag="h", bufs=8)
                        nc.vector.tensor_add(hf[:], t2p[:], zhp[:])
                        hN.append(hf)
                    hT = hN
                    hf_pending = None

                # ---- n gate: tanh(2^-12*(ghn + 2^12*b_hhn)*r-ish + b_ihn);
                # the (hn + b_hhn)*r combine is one DVE scalar_tensor_tensor ----
                t2_g = []
                for m in range(KE):
                    col = 2 * E + m * 128
                    ps = pp.tile([128, B], F32, name="ps_mm", tag="mm", bufs=8)
                    for p in range(PH):
                        nc.tensor.matmul(
                            ps[:],
                            w_hh[p][:, :, col : col + 128],
                            hq[p][:],
                            start=(p == 0),
                            stop=(p == PH - 1),
                            perf_mode=DR,
                        )
                    t2 = sb.tile([128, B], F32, name="t2", tag="t2", bufs=8)
                    nc.vector.scalar_tensor_tensor(
                        t2[:], ps[:], b_hhn12[:, m : m + 1], r_g[m][:],
                        op0=ALU.add, op1=ALU.mult,
                    )
                    t2_g.append(t2)

                if t > 0:
                    for m in range(KE):
                        col = 2 * E + m * 128
                        ps = pp.tile([128, B], F32, name="ps_mm", tag="mm", bufs=8)
                        for p in range(PH):
                            nc.tensor.matmul(
                                ps[:],
                                w_ih[p][:, :, col : col + 128],
                                embq[p][:],
                                start=(p == 0),
                                stop=(p == PH - 1),
                                perf_mode=DR,
                            )
                        nc.vector.tensor_add(t2_g[m][:], t2_g[m][:], ps[:])

                # z*h on the Pool engine, off the critical path
                zh_g = []
                for m in range(KE):
                    zh = sb.tile([128, B], F32, name="zh", tag="zh", bufs=8)
                    nc.gpsimd.tensor_mul(zh[:], z_g[m][:], hT[m][:])
                    zh_g.append(zh)

                # ---- h' = (1-z)*n + z*h; bf16 h2 (logits operand) written
                # first, fp8 h_q cast from h2, fp32 master deferred ----
                last = t == T - 1
                if not last:
                    hq = [
                        sb.tile([128, 2, B], FP8, name="hq", tag="hq", bufs=4)
                        for _ in range(PH)
                    ]
                h2 = []
                hf_pending = []
                for m in range(KE):
                    t2 = t2_g[m]
                    nc.scalar.activation(
                        t2[:], t2[:], AF.Tanh,
                        bias=b_ihn[:, m : m + 1], scale=SGI,
                    )  # n, in place
                    nc.vector.tensor_mul(t2[:], t2[:], omz_g[m][:])  # (1-z)*n
                    hm = sb.tile([128, B], BF16, name="h2", tag="h2", bufs=8)
                    nc.vector.tensor_add(hm[:], t2[:], zh_g[m][:])
                    h2.append(hm)
                    if not last:
                        nc.scalar.activation(
                            hq[m // 2][:, m % 2, :], hm[:], AF.Copy, scale=SH
                        )
                    hf_pending.append((t2, zh_g[m]))

                # dependency-free warm-up matmuls into the j=0 logits bank:
                # they run the moment the PE drains its real queue, keeping
                # the p-state up through the GRU-tail idle window.  The real
                # k=0 matmul below has start=True, which wipes the junk.
                ps0 = pp.tile([128, B], F32, name="ps_mm", tag="mm", bufs=8)
                for _ in range(12):
                    nc.tensor.matmul(
                        ps0[:],
                        w_e2d[1][:, 0:128],
                        w_e2d[0][:, 0:B],
                        start=True,
                        stop=True,
                    )

                # ---- logits psum (bf16 matmuls); exp -> fp8 eT pairs for the
                # next emb matmul; raw fp16 logits stream out (host adds bias
                # + softmax); rowsum via fp8 DR ones matmul -> rbc = 16/S ----
                if not last:
                    eTq = [
                        sb.tile([128, 2, B], FP8, name="eT", tag="eT", bufs=8)
                        for _ in range(PV)
                    ]
                    ps_s = pp.tile([128, B], F32, name="ps_s", tag="mm", bufs=8)
                for j in range(KV):
                    ps = ps0 if j == 0 else pp.tile(
                        [128, B], F32, name="ps_mm", tag="mm", bufs=8
                    )
                    for k in range(KE):
                        nc.tensor.matmul(
                            ps[:],
                            w_e2d[k][:, j * 128 : (j + 1) * 128],
                            h2[k][:],
                            start=(k == 0),
                            stop=(k == KE - 1),
                        )
                    if not last:
                        nc.scalar.activation(
                            eTq[j // 2][:, j % 2, :], ps[:], AF.Exp,
                            bias=b_e2d[:, j : j + 1],
                        )
                    lo = sb.tile([128, B], FP16, name="lo", tag="lo", bufs=16)
                    nc.vector.tensor_copy(lo[:], ps[:])
                    nc.sync.dma_start(out_l[t, j * 128 : (j + 1) * 128, :], lo[:])
                    if not last and j % 2 == 1:
                        jp = j // 2
                        nc.tensor.matmul(
                            ps_s[:],
                            ones8[:],
                            eTq[jp][:],
                            start=(jp == 0),
                            stop=(jp == PV - 1),
                            perf_mode=DR,
                        )
                if not last:
                    rbc = sb.tile([128, B], F32, name="rbc", tag="rbc", bufs=2)
                    nc.vector.reciprocal_approx_fast(rbc[:], ps_s[:])

    nc.compile()
    return nc


def _prep_inputs(x, W_d2e, W_ih, W_hh, b_ih, b_hh, W_e2d, b_e2d):
    import ml_dtypes

    E = x.shape[1]
    V = np.asarray(W_e2d).shape[0]
    KE = E // 128
    KV = V // 128
    PH = KE // 2
    PV = KV // 2
    E4 = ml_dtypes.float8_e4m3
    BFN = ml_dtypes.bfloat16

    def c(a, dt=np.float32):
        return np.ascontiguousarray(np.asarray(a, dtype=np.float32).astype(dt))

    def pairs(WT, scale, npairs):
        # WT: [contraction, cols] -> list of [128, 2, cols] fp8, pair p holds
        # contraction rows [p*256+i*128+k]
        cols = WT.shape[1]
        a = (np.asarray(WT, dtype=np.float32) * scale).reshape(
            npairs, 2, 128, cols
        )
        return [c(a[p].transpose(1, 0, 2), E4) for p in range(npairs)]

    b_ih = np.asarray(b_ih, dtype=np.float32)
    b_hh = np.asarray(b_hh, dtype=np.float32)
    brz_sum = (b_ih + b_hh)[: 2 * E].reshape(2 * KE, 128).T   # [128, 8]
    brz_negz = -(b_ih + b_hh)[E : 2 * E].reshape(KE, 128).T   # [128, 4]

    whh_p = pairs(np.asarray(W_hh).T, SWHH, PH)
    wih_p = pairs(np.asarray(W_ih).T, SWIH, PH)
    wd2e_p = pairs(np.asarray(W_d2e).T, SWD2E, PV)

    shared = {
        "we2dT": c(np.asarray(W_e2d).T, BFN),                  # [E, V]
        "brz": c(np.concatenate([brz_sum, brz_negz], axis=1)),  # [128, 12]
        "bihn": c(b_ih[2 * E :].reshape(KE, 128).T),
        "bhhn12": c(b_hh[2 * E :].reshape(KE, 128).T * 4096.0),
        "be2d": c(np.asarray(b_e2d).reshape(KV, 128).T),
    }
    for p in range(PH):
        shared[f"whh8_{p}"] = whh_p[p]
        shared[f"wih8_{p}"] = wih_p[p]
    for p in range(PV):
        shared[f"wd2e8_{p}"] = wd2e_p[p]

    N = x.shape[0]
    B = N // N_CORES
    in_maps = []
    for core in range(N_CORES):
        m = dict(shared)
        m["xT"] = c(np.asarray(x)[core * B : (core + 1) * B, :].T)  # [E, B]
        in_maps.append(m)
    return in_maps, B


def _run(inputs, trace=False):
    from concourse.bass_utils import run_bass_kernel_spmd

    x = np.asarray(inputs["x"], dtype=np.float32)
    T = int(inputs["max_len"])
    N, E = x.shape
    V = np.asarray(inputs["W_e2d"]).shape[0]
    assert N % N_CORES == 0 and E % 256 == 0 and V % 256 == 0

    in_maps, B = _prep_inputs(
        x,
        inputs["W_d2e"],
        inputs["W_ih"],
        inputs["W_hh"],
        inputs["b_ih"],
        inputs["b_hh"],
        inputs["W_e2d"],
        inputs["b_e2d"],
    )
    nc = _build(T, B, E, V)
    res = run_bass_kernel_spmd(
        nc, in_maps, core_ids=list(range(N_CORES)), trace=trace
    )

    b_e2d = np.asarray(inputs["b_e2d"], dtype=np.float32)
    full = np.empty((T, N, V), dtype=np.float32)
    for core in range(N_CORES):
        l = np.asarray(res.results[core]["out_l"], dtype=np.float32)  # [T, V, B]
        l += b_e2d[None, :, None]
        l -= l.max(axis=1, keepdims=True)
        e = np.exp(l)
        e /= e.sum(axis=1, keepdims=True)
        full[:, core * B : (core + 1) * B, :] = np.transpose(e, (0, 2, 1))
    return full, res


def kernel(**inputs):
    full, _ = _run(inputs, trace=False)
    return full


def run_traced(**inputs):
    return _run(inputs, trace=True)


# revision 19
# speedup vs baseline: 1.4777x; 1.2433x over previous
"""Trainium2 Bass kernel for a differentiable GRU decoder — fp8 DoubleRow version.

Per step t (max_len=32 steps), batch N=4096, E=512, V=1024:
    emb    = probs_{t-1} @ W_d2e.T            # [N, E]
    h      = GRUCell(emb, h)                  # [N, E]
    logits = h @ W_e2d.T + b_e2d              # [N, V]
    probs  = softmax(logits)                  # [N, V]  -> output[t]

Sharding: data-parallel over N across 8 cores (512 rows each), weights
replicated, the 32-step scan stays local per core — no collectives.

v2 design (vs the bf16 baseline):
- All matmuls except logits run as fp8(e4m3) DoubleRow (256-contraction
  per instruction, ~2x bf16 MAC throughput on HW).  Numerics validated
  in sim: logits path must stay bf16 (h2/W_e2d); everything else e4m3.
- Power-of-2 scales keep every fp8 tensor in e4m3's normal range AND
  make the gh and gx halves share one PSUM accumulation group:
      h_q = h*32, W_hh*128  -> gh psum scale 2^12
      emb_q = emb*256, W_ih*16 -> gx psum scale 2^12
  The sigmoid/tanh drains undo the scale via the ACT `scale` operand.
- The device streams RAW fp16 logits (no bias); the host adds b_e2d and
  does the softmax.  This removes the bf16-exp output floor (~4e-3) and
  the on-device out-normalization entirely.
- exp(logits) is written once, as the fp8 eT operand of the next emb
  matmul; the row-sum (partition reduction) is a fp8 DoubleRow matmul
  against a 2^-4 ones matrix, giving rbc = 16/S which folds the 2^8
  emb scale into the drain: emb_q = psum(=16*S*emb) * rbc = 256*emb.
- GRU tail keeps the baseline tricks: gh emitted before gx in each
  PSUM group, (1-z) via a negated sigmoid drain, z*h on the Pool
  engine off the critical path, fp32 h master deferred one step.
"""

import os
import sys
import types

import numpy as np

import concourse.bacc as bacc
import concourse.mybir as mybir
import concourse.tile as tile

F32 = mybir.dt.float32
BF16 = mybir.dt.bfloat16
FP16 = mybir.dt.float16
FP8 = mybir.dt.float8e4
AF = mybir.ActivationFunctionType
DR = mybir.MatmulPerfMode.DoubleRow
ALU = mybir.AluOpType

N_CORES = 8

SH = 32.0        # h_q scale (|h| <= ~5.1 early; 5.1*32 < 240)
SWHH = 128.0     # W_hh scale -> gh psum 2^12
SEMB = 256.0     # emb_q scale (|emb| <= 1/32; *256 <= 8)
SWIH = 16.0      # W_ih scale -> gx psum 2^12
SWD2E = 16.0     # W_d2e scale
SGI = 1.0 / 4096.0   # gate drain scale 2^-12
SONES = 1.0 / 16.0   # ones value: rowsum psum = S/16, rbc = 16/S


def _install_ntff_hook():
    """Register the axon NTFF profiling hook if the image's antenv lacks it."""
    try:
        import antenv.axon_hooks  # noqa: F401
        return
    except ImportError:
        pass
    try:
        from trn_agent_boot.trn_boot import _ntff_profile_via_ctypes

        hook = _ntff_profile_via_ctypes("/opt/axon/libaxon_pjrt.so")
    except Exception:
        hook = None
    mod = types.ModuleType("antenv.axon_hooks")
    mod.get_axon_ntff_profile_hook = lambda: hook
    mod.set_axon_ntff_profile_hook = lambda h: None
    sys.modules["antenv.axon_hooks"] = mod


_install_ntff_hook()


def _build(T, B, E, V):
    """Build the per-core Bacc module. B = per-core batch (free dim)."""
    KE = E // 128   # 4 E-blocks
    KV = V // 128   # 8 V-blocks
    PH = KE // 2    # 2 contraction pairs over E
    PV = KV // 2    # 4 contraction pairs over V

    nc = bacc.Bacc(None, target_bir_lowering=False)

    xT = nc.dram_tensor("xT", [E, B], F32, kind="ExternalInput")
    whh8 = [
        nc.dram_tensor(f"whh8_{p}", [128, 2, 3 * E], FP8, kind="ExternalInput")
        for p in range(PH)
    ]
    wih8 = [
        nc.dram_tensor(f"wih8_{p}", [128, 2, 3 * E], FP8, kind="ExternalInput")
        for p in range(PH)
    ]
    wd2e8 = [
        nc.dram_tensor(f"wd2e8_{p}", [128, 2, E], FP8, kind="ExternalInput")
        for p in range(PV)
    ]
    we2dT = nc.dram_tensor("we2dT", [E, V], BF16, kind="ExternalInput")
    # b_rz: cols [0:2*KE) = (b_ih+b_hh) for r,z; cols [2*KE:3*KE) = negated z part
    brz = nc.dram_tensor("brz", [128, 3 * KE], F32, kind="ExternalInput")
    bihn = nc.dram_tensor("bihn", [128, KE], F32, kind="ExternalInput")
    bhhn12 = nc.dram_tensor("bhhn12", [128, KE], F32, kind="ExternalInput")
    be2d = nc.dram_tensor("be2d", [128, KV], F32, kind="ExternalInput")
    # raw fp16 logits, NO bias; host adds b_e2d and softmaxes
    out_l = nc.dram_tensor("out_l", [T, V, B], FP16, kind="ExternalOutput")

    with tile.TileContext(nc) as tc:
        with (
            tc.tile_pool(name="w", bufs=1) as wp,
            tc.tile_pool(name="sb", bufs=1) as sb,
            tc.tile_pool(name="ps", bufs=1, space="PSUM") as pp,
        ):
            # ---- initial state h = x, loaded first (t=0 gh needs h_q);
            # x rides the SWDGE queues to avoid serializing on weight DMAs
            hT = []
            for m in range(KE):
                hf = sb.tile([128, B], F32, name="h", tag="h", bufs=8)
                nc.gpsimd.dma_start(hf[:], xT[m * 128 : (m + 1) * 128, :])
                hT.append(hf)
            hq = [
                sb.tile([128, 2, B], FP8, name="hq", tag="hq", bufs=4)
                for _ in range(PH)
            ]
            for m in range(KE):
                nc.scalar.activation(
                    hq[m // 2][:, m % 2, :], hT[m][:], AF.Copy, scale=SH
                )

            # ---- persistent weights, in first-use order ----
            w_hh = []
            for p in range(PH):
                wt = wp.tile([128, 2, 3 * E], FP8, name=f"w_hh{p}", tag=f"w_hh{p}")
                nc.sync.dma_start(wt[:], whh8[p][:])
                w_hh.append(wt)
            b_rz = wp.tile([128, 3 * KE], F32, name="b_rz", tag="b_rz")
            nc.sync.dma_start(b_rz[:], brz[:])
            b_ihn = wp.tile([128, KE], F32, name="b_ihn", tag="b_ihn")
            nc.sync.dma_start(b_ihn[:], bihn[:])
            b_hhn12 = wp.tile([128, KE], F32, name="b_hhn12", tag="b_hhn12")
            nc.sync.dma_start(b_hhn12[:], bhhn12[:])
            b_e2d = wp.tile([128, KV], F32, name="b_e2d", tag="b_e2d")
            nc.sync.dma_start(b_e2d[:], be2d[:])
            w_e2d = []
            for k in range(KE):
                wt = wp.tile([128, V], BF16, name=f"w_e2d{k}", tag=f"w_e2d{k}")
                nc.sync.dma_start(wt[:], we2dT[k * 128 : (k + 1) * 128, :])
                w_e2d.append(wt)
            w_d2e = []
            for p in range(PV):
                wt = wp.tile([128, 2, E], FP8, name=f"w_d2e{p}", tag=f"w_d2e{p}")
                nc.sync.dma_start(wt[:], wd2e8[p][:])
                w_d2e.append(wt)
            w_ih = []
            for p in range(PH):
                wt = wp.tile([128, 2, 3 * E], FP8, name=f"w_ih{p}", tag=f"w_ih{p}")
                nc.sync.dma_start(wt[:], wih8[p][:])
                w_ih.append(wt)

            ones_f32 = wp.tile([128, 2, 128], F32, name="ones_f32", tag="ones_f32")
            nc.gpsimd.memset(ones_f32[:], SONES)
            ones8 = wp.tile([128, 2, 128], FP8, name="ones8", tag="ones8")
            nc.scalar.activation(ones8[:], ones_f32[:], AF.Copy)

            eTq = None       # fp8 exp pairs of previous step
            rbc = None       # 16/rowsum broadcast [128, B] fp32
            hf_pending = None  # (t2, zh) pairs for the deferred fp32 h master

            for t in range(T):
                # ---- emb_q = (softmax_{t-1} @ W_d2e.T) * 256, via fp8 DR;
                # normalization + scales folded into the rbc drain ----
                embq = None
                if t > 0:
                    embq = []
                    for half in range(PH):
                        e16 = sb.tile(
                            [128, 2 * B], BF16, name="emb16", tag="emb16", bufs=4
                        )
                        for ml in range(2):
                            m = half * 2 + ml
                            ps = pp.tile(
                                [128, B], F32, name="ps_mm", tag="mm", bufs=8
                            )
                            for jp in range(PV):
                                nc.tensor.matmul(
                                    ps[:],
                                    w_d2e[jp][:, :, m * 128 : (m + 1) * 128],
                                    eTq[jp][:],
                                    start=(jp == 0),
                                    stop=(jp == PV - 1),
                                    perf_mode=DR,
                                )
                            nc.vector.tensor_mul(
                                e16[:, ml * B : (ml + 1) * B], ps[:], rbc[:]
                            )
                        eq = sb.tile(
                            [128, 2, B], FP8, name="embq", tag="embq", bufs=4
                        )
                        nc.scalar.activation(
                            eq[:], e16[:].rearrange("p (a b) -> p a b", a=2),
                            AF.Copy,
                        )
                        embq.append(eq)

                # ---- gates r, z: sigmoid(2^-12 psum + b); gh pairs emitted
                # first so the PE has emb-independent work during the tail.
                # z additionally drains (1-z) via sigmoid(-x, -b) ----
                r_g = []
                z_g = []
                omz_g = []
                for g in range(2):
                    for m in range(KE):
                        col = g * E + m * 128
                        ps = pp.tile([128, B], F32, name="ps_mm", tag="mm", bufs=8)
                        for p in range(PH):
                            nc.tensor.matmul(
                                ps[:],
                                w_hh[p][:, :, col : col + 128],
                                hq[p][:],
                                start=(p == 0),
                                stop=(t == 0 and p == PH - 1),
                                perf_mode=DR,
                            )
                        if t > 0:
                            for p in range(PH):
                                nc.tensor.matmul(
                                    ps[:],
                                    w_ih[p][:, :, col : col + 128],
                                    embq[p][:],
                                    start=False,
                                    stop=(p == PH - 1),
                                    perf_mode=DR,
                                )
                        j = g * KE + m
                        if g == 0:
                            gt = sb.tile(
                                [128, B], F32, name="gate_r", tag="gate_r", bufs=4
                            )
                            nc.scalar.activation(
                                gt[:], ps[:], AF.Sigmoid,
                                bias=b_rz[:, j : j + 1], scale=SGI,
                            )
                            r_g.append(gt)
                        else:
                            zt = sb.tile(
                                [128, B], F32, name="gate_z", tag="gate_z", bufs=4
                            )
                            nc.scalar.activation(
                                zt[:], ps[:], AF.Sigmoid,
                                bias=b_rz[:, j : j + 1], scale=SGI,
                            )
                            oz = sb.tile(
                                [128, B], F32, name="gate_omz", tag="gate_omz",
                                bufs=4,
                            )
                            nj = 2 * KE + m
                            nc.scalar.activation(
                                oz[:], ps[:], AF.Sigmoid,
                                bias=b_rz[:, nj : nj + 1], scale=-SGI,
                            )
                            omz_g.append(oz)
                            z_g.append(zt)

                # fp32 h master of the PREVIOUS step, deferred past the gate
                # matmuls so their hoisted waits never include these DVE ops
                if hf_pending is not None:
                    hN = []
                    for m in range(KE):
                        t2p, zhp = hf_pending[m]
                        hf = sb.tile([128, B], F32, name="h", tag="h", bufs=8)
                        nc.vector.tensor_add(hf[:], t2p[:], zhp[:])
                        hN.append(hf)
                    hT = hN
                    hf_pending = None

                # ---- n gate: tanh(2^-12*(ghn + 2^12*b_hhn)*r-ish + b_ihn);
                # the (hn + b_hhn)*r combine is one DVE scalar_tensor_tensor ----
                t2_g = []
                for m in range(KE):
                    col = 2 * E + m * 128
                    ps = pp.tile([128, B], F32, name="ps_mm", tag="mm", bufs=8)
                    for p in range(PH):
                        nc.tensor.matmul(
                            ps[:],
                            w_hh[p][:, :, col : col + 128],
                            hq[p][:],
                            start=(p == 0),
                            stop=(p == PH - 1),
                            perf_mode=DR,
                        )
                    t2 = sb.tile([128, B], F32, name="t2", tag="t2", bufs=8)
                    nc.vector.scalar_tensor_tensor(
                        t2[:], ps[:], b_hhn12[:, m : m + 1], r_g[m][:],
                        op0=ALU.add, op1=ALU.mult,
                    )
                    t2_g.append(t2)

                if t > 0:
                    for m in range(KE):
                        col = 2 * E + m * 128
                        ps = pp.tile([128, B], F32, name="ps_mm", tag="mm", bufs=8)
                        for p in range(PH):
                            nc.tensor.matmul(
                                ps[:],
                                w_ih[p][:, :, col : col + 128],
                                embq[p][:],
                                start=(p == 0),
                                stop=(p == PH - 1),
                                perf_mode=DR,
                            )
                        nc.vector.tensor_add(t2_g[m][:], t2_g[m][:], ps[:])

                # z*h on the Pool engine, off the critical path
                zh_g = []
                for m in range(KE):
                    zh = sb.tile([128, B], F32, name="zh", tag="zh", bufs=8)
                    nc.gpsimd.tensor_mul(zh[:], z_g[m][:], hT[m][:])
                    zh_g.append(zh)

                # ---- h' = (1-z)*n + z*h; bf16 h2 (logits operand) written
                # first, fp8 h_q cast from h2, fp32 master deferred ----
                last = t == T - 1
                if not last:
                    hq = [
                        sb.tile([128, 2, B], FP8, name="hq", tag="hq", bufs=4)
                        for _ in range(PH)
                    ]
                h2 = []
                hf_pending = []
                for m in range(KE):
                    t2 = t2_g[m]
                    nc.scalar.activation(
                        t2[:], t2[:], AF.Tanh,
                        bias=b_ihn[:, m : m + 1], scale=SGI,
                    )  # n, in place
                    nc.vector.tensor_mul(t2[:], t2[:], omz_g[m][:])  # (1-z)*n
                    hm = sb.tile([128, B], BF16, name="h2", tag="h2", bufs=8)
                    nc.vector.tensor_add(hm[:], t2[:], zh_g[m][:])
                    h2.append(hm)
                    if not last:
                        nc.scalar.activation(
                            hq[m // 2][:, m % 2, :], hm[:], AF.Copy, scale=SH
                        )
                    hf_pending.append((t2, zh_g[m]))

                # dependency-free warm-up matmuls into the j=0 logits bank:
                # they run the moment the PE drains its real queue, keeping
                # the p-state up through the GRU-tail idle window.  The real
                # k=0 matmul below has start=True, which wipes the junk.
                ps0 = pp.tile([128, B], F32, name="ps_mm", tag="mm", bufs=8)
                for _ in range(12):
                    nc.tensor.matmul(
                        ps0[:],
                        w_e2d[1][:, 0:128],
                        w_e2d[0][:, 0:B],
                        start=True,
                        stop=True,
                    )

                # ---- logits psum (bf16 matmuls); exp -> fp8 eT pairs for the
                # next emb matmul; raw fp16 logits stream out (host adds bias
                # + softmax); rowsum via fp8 DR ones matmul -> rbc = 16/S ----
                if not last:
                    eTq = [
                        sb.tile([128, 2, B], FP8, name="eT", tag="eT", bufs=8)
                        for _ in range(PV)
                    ]
                    ps_s = pp.tile([128, B], F32, name="ps_s", tag="mm", bufs=8)
                for j in range(KV):
                    ps = ps0 if j == 0 else pp.tile(
                        [128, B], F32, name="ps_mm", tag="mm", bufs=8
                    )
                    for k in range(KE):
                        nc.tensor.matmul(
                            ps[:],
                            w_e2d[k][:, j * 128 : (j + 1) * 128],
                            h2[k][:],
                            start=(k == 0),
                            stop=(k == KE - 1),
                        )
                    if not last:
                        nc.scalar.activation(
                            eTq[j // 2][:, j % 2, :], ps[:], AF.Exp,
                            bias=b_e2d[:, j : j + 1],
                        )
                    lo = sb.tile([128, B], FP16, name="lo", tag="lo", bufs=16)
                    nc.vector.tensor_copy(lo[:], ps[:])
                    nc.sync.dma_start(out_l[t, j * 128 : (j + 1) * 128, :], lo[:])
                # rowsum matmuls trail the whole j-loop: interleaved, each
                # waits its exp pair and head-of-line-blocks the next (ready)
                # j-group's matmuls on the in-order PE queue
                if not last:
                    for jp in range(PV):
                        nc.tensor.matmul(
                            ps_s[:],
                            ones8[:],
                            eTq[jp][:],
                            start=(jp == 0),
                            stop=(jp == PV - 1),
                            perf_mode=DR,
                        )
                if not last:
                    rbc = sb.tile([128, B], F32, name="rbc", tag="rbc", bufs=2)
                    nc.vector.reciprocal_approx_fast(rbc[:], ps_s[:])

    nc.compile()
    return nc


def _prep_inputs(x, W_d2e, W_ih, W_hh, b_ih, b_hh, W_e2d, b_e2d):
    import ml_dtypes

    E = x.shape[1]
    V = np.asarray(W_e2d).shape[0]
    KE = E // 128
    KV = V // 128
    PH = KE // 2
    PV = KV // 2
    E4 = ml_dtypes.float8_e4m3
    BFN = ml_dtypes.bfloat16

    def c(a, dt=np.float32):
        return np.ascontiguousarray(np.asarray(a, dtype=np.float32).astype(dt))

    def pairs(WT, scale, npairs):
        # WT: [contraction, cols] -> list of [128, 2, cols] fp8, pair p holds
        # contraction rows [p*256+i*128+k]
        cols = WT.shape[1]
        a = (np.asarray(WT, dtype=np.float32) * scale).reshape(
            npairs, 2, 128, cols
        )
        return [c(a[p].transpose(1, 0, 2), E4) for p in range(npairs)]

    b_ih = np.asarray(b_ih, dtype=np.float32)
    b_hh = np.asarray(b_hh, dtype=np.float32)
    brz_sum = (b_ih + b_hh)[: 2 * E].reshape(2 * KE, 128).T   # [128, 8]
    brz_negz = -(b_ih + b_hh)[E : 2 * E].reshape(KE, 128).T   # [128, 4]

    whh_p = pairs(np.asarray(W_hh).T, SWHH, PH)
    wih_p = pairs(np.asarray(W_ih).T, SWIH, PH)
    wd2e_p = pairs(np.asarray(W_d2e).T, SWD2E, PV)

    shared = {
        "we2dT": c(np.asarray(W_e2d).T, BFN),                  # [E, V]
        "brz": c(np.concatenate([brz_sum, brz_negz], axis=1)),  # [128, 12]
        "bihn": c(b_ih[2 * E :].reshape(KE, 128).T),
        "bhhn12": c(b_hh[2 * E :].reshape(KE, 128).T * 4096.0),
        "be2d": c(np.asarray(b_e2d).reshape(KV, 128).T),
    }
    for p in range(PH):
        shared[f"whh8_{p}"] = whh_p[p]
        shared[f"wih8_{p}"] = wih_p[p]
    for p in range(PV):
        shared[f"wd2e8_{p}"] = wd2e_p[p]

    N = x.shape[0]
    B = N // N_CORES
    in_maps = []
    for core in range(N_CORES):
        m = dict(shared)
        m["xT"] = c(np.asarray(x)[core * B : (core + 1) * B, :].T)  # [E, B]
        in_maps.append(m)
    return in_maps, B


def _run(inputs, trace=False):
    from concourse.bass_utils import run_bass_kernel_spmd

    x = np.asarray(inputs["x"], dtype=np.float32)
    T = int(inputs["max_len"])
    N, E = x.shape
    V = np.asarray(inputs["W_e2d"]).shape[0]
    assert N % N_CORES == 0 and E % 256 == 0 and V % 256 == 0

    in_maps, B = _prep_inputs(
        x,
        inputs["W_d2e"],
        inputs["W_ih"],
        inputs["W_hh"],
        inputs["b_ih"],
        inputs["b_hh"],
        inputs["W_e2d"],
        inputs["b_e2d"],
    )
    nc = _build(T, B, E, V)
    res = run_bass_kernel_spmd(
        nc, in_maps, core_ids=list(range(N_CORES)), trace=trace
    )

    b_e2d = np.asarray(inputs["b_e2d"], dtype=np.float32)
    full = np.empty((T, N, V), dtype=np.float32)
    for core in range(N_CORES):
        l = np.asarray(res.results[core]["out_l"], dtype=np.float32)  # [T, V, B]
        l += b_e2d[None, :, None]
        l -= l.max(axis=1, keepdims=True)
        e = np.exp(l)
        e /= e.sum(axis=1, keepdims=True)
        full[:, core * B : (core + 1) * B, :] = np.transpose(e, (0, 2, 1))
    return full, res


def kernel(**inputs):
    full, _ = _run(inputs, trace=False)
    return full


def run_traced(**inputs):
    return _run(inputs, trace=True)
